# revision 5
# baseline (speedup 1.0000x reference)
"""Causal self-attention (B=4, T=2048, C=2048, H=16) on 8 trn2 NeuronCores.

Sharding: tensor-parallel over heads — 2 heads per core. Every core gets the
full (pre-transposed) activation xT, its 2 heads' slice of Wqkv columns and
Wproj rows, computes a full [B*T, C] partial output (fp16), and the host sums
the 8 partials (the "all-reduce after output projection" done host-side).

Per-core dataflow (all matmuls on PE, fp16 operands):
  xT tiles --DMA--> QKV proj -> Q^T,K^T [d,t] + V [t,d] (via PE transpose)
  S^T = K^T-block.T @ Q^T chunks (PSUM f32) -> exp (ACT) -> P^T fp16 in SBUF
  causal mask: GPSIMD affine_select zeroes the lower triangle of diag blocks
  den = ones^T @ P^T blocks (PE, PSUM acc); y^T = sum_k V_k @ P^T_k (PE)
  rec = exp(-ln(den)) on ACT (both funcs in one table set);
  broadcast rec via PE, scale y^T on DVE  (deferred past independent QKV work
  so the PE FIFO never stalls on the recip chain)
  out_partial = y^T.T @ Wproj-rows -> fp16 -> DMA out

Scheduling: next batch's QKV-projection matmuls are drip-fed between the
attention blocks' dependent matmuls (score depth-2 pipeline) so the PE always
has runnable work during ACT exp latency; long PE-idle gaps would re-throttle
the PE clock (HAM) to 1.2 GHz.
"""
import numpy as np

B, T, C = 4, 2048, 2048
H, HD = 16, 128
N_CORES = 8
HPC = H // N_CORES          # heads per core = 2
SCALE = float(1.0 / np.sqrt(HD))

_CACHE = {}


def _build_nc():
    import concourse.bass as bass
    from concourse import bacc
    import concourse.tile as tile
    import concourse.mybir as mybir
    from concourse.masks import make_identity
    from contextlib import ExitStack

    f32 = mybir.dt.float32
    f16 = mybir.dt.float16
    Exp = mybir.ActivationFunctionType.Exp
    Ln = mybir.ActivationFunctionType.Ln

    nc = bacc.Bacc("TRN2", target_bir_lowering=False, debug=False,
                   enable_asserts=True, num_devices=N_CORES)

    # Inputs (per-core shards prepared on host)
    xT = nc.dram_tensor("xt", [C, B * T], f16, kind="ExternalInput").ap()
    wqkv = nc.dram_tensor("wqkv", [C, 6 * HD], f16, kind="ExternalInput").ap()
    wproj = nc.dram_tensor("wproj", [HPC * HD, C], f16, kind="ExternalInput").ap()
    out = nc.dram_tensor("out", [B * T, C], f16, kind="ExternalOutput").ap()

    # DRAM views: c-chunked weights
    wqkv_v = wqkv.rearrange("(cc p) (jj d) -> p cc jj d", p=128, d=HD)  # [128,16,6,128]
    wproj_v = wproj.rearrange("(jh p) c -> p jh c", p=128)              # [128,2,2048]

    NCC = C // 128        # 16 contraction chunks
    NTCH = T // 512       # 4 t-chunks per batch

    with tile.TileContext(nc) as tc, ExitStack() as ctx:
        const = ctx.enter_context(tc.tile_pool(name="const", bufs=1))
        wpool = ctx.enter_context(tc.tile_pool(name="w", bufs=1))
        xtp = ctx.enter_context(tc.tile_pool(name="xt", bufs=2))
        qkvp = ctx.enter_context(tc.tile_pool(name="qkv", bufs=2))
        dnp = ctx.enter_context(tc.tile_pool(name="dn", bufs=4))
        rp = ctx.enter_context(tc.tile_pool(name="r", bufs=2))
        ptp = ctx.enter_context(tc.tile_pool(name="pt", bufs=2))
        ytp = ctx.enter_context(tc.tile_pool(name="yt", bufs=2))
        op = ctx.enter_context(tc.tile_pool(name="o", bufs=6))
        # PSUM: st 2 + qkv-acc 1 + transpose 1 + (yt_ps|o_ps) 2 + (den|rec) 2 = 8
        psA = ctx.enter_context(tc.tile_pool(name="psA", bufs=2, space="PSUM"))
        psQ = ctx.enter_context(tc.tile_pool(name="psQ", bufs=1, space="PSUM"))
        psV = ctx.enter_context(tc.tile_pool(name="psV", bufs=2, space="PSUM"))
        psD = ctx.enter_context(tc.tile_pool(name="psD", bufs=2, space="PSUM"))
        psT = ctx.enter_context(tc.tile_pool(name="psT", bufs=1, space="PSUM"))

        ident_f = const.tile([128, 128], f32)
        make_identity(nc, ident_f)
        ident_h = const.tile([128, 128], f16)
        nc.scalar.copy(ident_h, ident_f)
        ones_col = const.tile([128, 1], f16)
        nc.vector.memset(ones_col, 1.0)
        ones_row = const.tile([1, 128], f16)
        nc.vector.memset(ones_row, 1.0)

        w_sb = wpool.tile([128, NCC, 6, HD], f16)
        wp_sb = wpool.tile([128, 2, C], f16)

        def emit_qkv_dma(b, tch):
            t0 = b * T + tch * 512
            xt_t = xtp.tile([128, NCC, 512], f16, tag="xt")
            for cc in range(NCC):
                nc.sync.dma_start(
                    xt_t[:, cc, :], xT[cc * 128:(cc + 1) * 128, t0:t0 + 512])
            return xt_t

        def make_qkv_steps(qkv_tiles, xt_t, tch, qtag):
            """QKV-projection work for one 512-token chunk, as a list of
            small closures so it can be drip-fed between attention matmuls."""
            qt, kt, vt, v = qkv_tiles
            steps = []
            acc = {}

            def mm_step(jj, cc):
                if cc == 0:
                    pool = psA if qtag == "psA" else psQ
                    acc[jj] = pool.tile([128, 512], f32, tag=qtag,
                                        name="qk_ps")
                nc.tensor.matmul(acc[jj], w_sb[:, cc, jj, :], xt_t[:, cc, :],
                                 start=(cc == 0), stop=(cc == NCC - 1))
                if cc == NCC - 1:
                    dst = (qt, qt, kt, kt, vt, vt)[jj]
                    nc.vector.tensor_copy(
                        dst[:, jj % 2, tch * 512:(tch + 1) * 512], acc[jj])

            def tr_step(hh, tb):
                tg = tch * 4 + tb
                vp = psT.tile([128, 128], f16, tag="psT")
                nc.tensor.transpose(
                    vp, vt[:, hh, tg * 128:(tg + 1) * 128], ident_h)
                nc.vector.tensor_copy(v[:, tg, hh * HD:(hh + 1) * HD], vp)

            for jj in range(6):  # q_h0, q_h1, k_h0, k_h1, v_h0, v_h1
                for cc in range(NCC):
                    steps.append(lambda jj=jj, cc=cc: mm_step(jj, cc))
            for hh in range(HPC):
                for tb in range(4):
                    steps.append(lambda hh=hh, tb=tb: tr_step(hh, tb))
            return steps

        class Feeder:
            def __init__(self, steps):
                self.steps = steps
                self.i = 0

            def take(self, n):
                for _ in range(n):
                    if self.i < len(self.steps):
                        self.steps[self.i]()
                        self.i += 1

            def flush(self):
                self.take(len(self.steps) - self.i)

        def emit_attn_unit(b, qg, h, qkv_tiles, feeder):
            qt, kt, vt, v = qkv_tiles
            pt_sb = ptp.tile([128, T // 128, 512], f16, tag="pt")
            den_ps = psD.tile([1, 512], f32, tag="psD")
            yt_ps = psV.tile([128, 512], f32, tag="psV")
            nkb = 4 * qg + 4

            def score(kb):
                kk = kb - 4 * qg
                qs = max(0, kk) * 128
                st = psA.tile([128, 512], f32, tag="psA")
                nc.tensor.matmul(
                    st[:, qs:512], kt[:, h, kb * 128:(kb + 1) * 128],
                    qt[:, h, qg * 512 + qs:(qg + 1) * 512],
                    start=True, stop=True)
                return st, qs

            pipe = score(0)
            for kb in range(nkb):
                st, qs = pipe
                nc.scalar.activation(
                    pt_sb[:, kb, qs:512], st[:, qs:512], Exp, scale=SCALE)
                if kb - 4 * qg >= 0:
                    # causal mask: zero the lower triangle of the 128x128
                    # diagonal block (partition k_rel > free q_rel)
                    nc.gpsimd.affine_select(
                        out=pt_sb[:, kb, qs:qs + 128],
                        in_=pt_sb[:, kb, qs:qs + 128],
                        compare_op=mybir.AluOpType.is_ge, fill=0.0,
                        base=0, pattern=[[1, 128]], channel_multiplier=-1)
                if kb + 1 < nkb:
                    pipe = score(kb + 1)    # PE fills ACT-exp latency
                feeder.take(1)              # drip one QKV-projection step
                nc.tensor.matmul(
                    den_ps[0:1, qs:512], ones_col, pt_sb[:, kb, qs:512],
                    start=(kb == 0), stop=(kb == nkb - 1))
                nc.tensor.matmul(
                    yt_ps[:, qs:512], v[:, kb, h * HD:(h + 1) * HD],
                    pt_sb[:, kb, qs:512],
                    start=(kb == 0), stop=(kb == nkb - 1))
            # reciprocal on DVE ([1,512] costs ~3.3us there, but it runs
            # while later units / the QKV flush keep the PE fed)
            rec_row = dnp.tile([1, 512], f32, tag="rec")
            nc.vector.reciprocal(rec_row, den_ps[0:1, :])
            rec16 = dnp.tile([1, 512], f16, tag="rec16")
            nc.scalar.copy(rec16, rec_row)
            return rec16, yt_ps

        def emit_attn_norm(yt, h, state):
            rec16, yt_ps = state
            r_ps = psD.tile([128, 512], f32, tag="psD")
            nc.tensor.matmul(r_ps, ones_row, rec16, start=True, stop=True)
            r_sb = rp.tile([128, 512], f32, tag="rsb")
            nc.scalar.copy(r_sb, r_ps)
            nc.vector.tensor_mul(yt[:, h, :], yt_ps, r_sb)

        def emit_proj(b, qg, yt):
            for tt in range(4):
                for co in range(4):
                    o_ps = psV.tile([128, 512], f32, tag="psV")
                    for jh in range(HPC):
                        nc.tensor.matmul(
                            o_ps, yt[:, jh, tt * 128:(tt + 1) * 128],
                            wp_sb[:, jh, co * 512:(co + 1) * 512],
                            start=(jh == 0), stop=(jh == HPC - 1))
                    o_sb = op.tile([128, 512], f16, tag="osb")
                    nc.vector.tensor_copy(o_sb, o_ps)
                    r0 = b * T + qg * 512 + tt * 128
                    nc.sync.dma_start(
                        out[r0:r0 + 128, co * 512:(co + 1) * 512], o_sb)

        def alloc_qkv_tiles():
            qt = qkvp.tile([128, HPC, T], f16, tag="qt")
            kt = qkvp.tile([128, HPC, T], f16, tag="kt")
            vt = qkvp.tile([128, HPC, T], f16, tag="vt")
            v = qkvp.tile([128, T // 128, HPC * HD], f16, tag="v")
            return (qt, kt, vt, v)

        # Prologue: batch 0's QKV, with weight DMAs interleaved with the
        # first x chunk's DMAs so the first matmul starts ~1us in.
        tiles = alloc_qkv_tiles()
        xt0 = xtp.tile([128, NCC, 512], f16, tag="xt")
        for cc in range(NCC):
            nc.sync.dma_start(w_sb[:, cc], wqkv_v[:, cc])
            nc.sync.dma_start(xt0[:, cc, :], xT[cc * 128:(cc + 1) * 128, 0:512])
        nc.sync.dma_start(wp_sb, wproj_v)
        for s in make_qkv_steps(tiles, xt0, 0, "psA"):
            s()
        for tch in range(1, NTCH):
            xt_t = emit_qkv_dma(0, tch)
            for s in make_qkv_steps(tiles, xt_t, tch, "psA"):
                s()

        # chunk k (k=0..11) = batch k//4+1, t-chunk k%4; consumed at group k,
        # its xt DMA prefetched one group earlier so the loads never contend
        # with the previous group's output writes.
        pending_xt = emit_qkv_dma(1, 0)
        for b in range(B):
            nxt = alloc_qkv_tiles() if b + 1 < B else None
            for qg in range(4):
                k = 4 * b + qg
                if nxt is not None:
                    xt_t = pending_xt
                    if k + 1 < 4 * (B - 1):
                        pending_xt = emit_qkv_dma((k + 1) // 4 + 1, (k + 1) % 4)
                    feeder = Feeder(make_qkv_steps(nxt, xt_t, qg, "psQ"))
                else:
                    feeder = Feeder([])
                yt = ytp.tile([128, HPC, 512], f16, tag="yt")
                s0 = emit_attn_unit(b, qg, 0, tiles, feeder)
                s1 = emit_attn_unit(b, qg, 1, tiles, feeder)
                emit_attn_norm(yt, 0, s0)
                feeder.flush()      # bulk of next batch's QKV hides the
                emit_attn_norm(yt, 1, s1)  # recip->broadcast->scale chains
                emit_proj(b, qg, yt)
            tiles = nxt

    nc.compile()
    return nc


def _get_nc():
    if "nc" not in _CACHE:
        _CACHE["nc"] = _build_nc()
    return _CACHE["nc"]


def _make_in_maps(x2d, Wqkv, Wproj):
    xT = np.ascontiguousarray(x2d.T).astype(np.float16)  # [C, B*T]
    in_maps = []
    for c in range(N_CORES):
        h0 = c * HPC
        cols = []
        for part in range(3):  # q, k, v blocks of Wqkv columns
            for h in range(HPC):
                j0 = part * C + (h0 + h) * HD
                cols.append(Wqkv[:, j0:j0 + HD])
        wq = np.ascontiguousarray(np.concatenate(cols, axis=1)).astype(np.float16)
        wp = np.ascontiguousarray(
            Wproj[h0 * HD:(h0 + HPC) * HD, :]).astype(np.float16)
        in_maps.append({"xt": xT, "wqkv": wq, "wproj": wp})
    return in_maps


def run_shards(in_maps, trace=False):
    from concourse.bass_utils import run_bass_kernel_spmd
    nc = _get_nc()
    last_err = None
    for _attempt in range(3):
        try:
            return run_bass_kernel_spmd(
                nc, in_maps, core_ids=list(range(N_CORES)), trace=trace)
        except Exception as e:  # transient NRT device errors — retry
            last_err = e
            if "UNAVAILABLE" not in str(e) and "UNRECOVERABLE" not in str(e):
                raise
    raise last_err


def kernel(x, Wqkv, Wproj):
    x = np.asarray(x, dtype=np.float32)
    Wqkv = np.asarray(Wqkv, dtype=np.float32)
    Wproj = np.asarray(Wproj, dtype=np.float32)
    x2d = np.ascontiguousarray(x.reshape(B * T, C))

    in_maps = _make_in_maps(x2d, Wqkv, Wproj)
    res = run_shards(in_maps)

    acc = res.results[0]["out"].astype(np.float32)
    for c in range(1, N_CORES):
        acc += res.results[c]["out"].astype(np.float32)
    return acc.reshape(B, T, C)


# revision 11
# speedup vs baseline: 1.0166x; 1.0166x over previous
"""Causal self-attention (B=4, T=2048, C=2048, H=16) on 8 trn2 NeuronCores.

Sharding: tensor-parallel over heads — 2 heads per core. Every core gets the
full (pre-transposed) activation xT, its 2 heads' slice of Wqkv columns and
Wproj rows, computes a full [B*T, C] partial output (fp16), and the host sums
the 8 partials (the "all-reduce after output projection" done host-side).

Per-core dataflow (all matmuls on PE, fp16 operands):
  xT tiles --DMA--> QKV proj -> Q^T,K^T [d,t] + V [t,d] (via PE transpose)
  S^T = K^T-block.T @ Q^T chunks (PSUM f32) -> exp (ACT) -> P^T fp16 in SBUF
  causal mask: GPSIMD affine_select zeroes the lower triangle of diag blocks
  den = ones^T @ P^T blocks (PE, PSUM acc); y^T = sum_k V_k @ P^T_k (PE)
  rec = exp(-ln(den)) on ACT (both funcs in one table set);
  broadcast rec via PE, scale y^T on DVE  (deferred past independent QKV work
  so the PE FIFO never stalls on the recip chain)
  out_partial = y^T.T @ Wproj-rows -> fp16 -> DMA out

Scheduling: next batch's QKV-projection matmuls are drip-fed between the
attention blocks' dependent matmuls (score depth-2 pipeline) so the PE always
has runnable work during ACT exp latency; long PE-idle gaps would re-throttle
the PE clock (HAM) to 1.2 GHz.
"""
import numpy as np

B, T, C = 4, 2048, 2048
H, HD = 16, 128
N_CORES = 8
HPC = H // N_CORES          # heads per core = 2
SCALE = float(1.0 / np.sqrt(HD))

_CACHE = {}


def _build_nc():
    import concourse.bass as bass
    from concourse import bacc
    import concourse.tile as tile
    import concourse.mybir as mybir
    from contextlib import ExitStack

    f32 = mybir.dt.float32
    f16 = mybir.dt.float16
    Exp = mybir.ActivationFunctionType.Exp

    nc = bacc.Bacc("TRN2", target_bir_lowering=False, debug=False,
                   enable_asserts=True, num_devices=N_CORES)

    # Inputs (per-core shards prepared on host)
    xT = nc.dram_tensor("xt", [C, B * T], f16, kind="ExternalInput").ap()
    wqkv = nc.dram_tensor("wqkv", [C, 6 * HD], f16, kind="ExternalInput").ap()
    wproj = nc.dram_tensor("wproj", [HPC * HD, C], f16, kind="ExternalInput").ap()
    out = nc.dram_tensor("out", [B * T, C], f16, kind="ExternalOutput").ap()

    # DRAM views: c-chunked weights
    wqkv_v = wqkv.rearrange("(cc p) (jj d) -> p cc jj d", p=128, d=HD)  # [128,16,6,128]
    wproj_v = wproj.rearrange("(jh p) c -> p jh c", p=128)              # [128,2,2048]

    NCC = C // 128        # 16 contraction chunks
    NTCH = T // 512       # 4 t-chunks per batch

    with tile.TileContext(nc) as tc, ExitStack() as ctx:
        const = ctx.enter_context(tc.tile_pool(name="const", bufs=1))
        wpool = ctx.enter_context(tc.tile_pool(name="w", bufs=1))
        xtp = ctx.enter_context(tc.tile_pool(name="xt", bufs=2))
        qkvp = ctx.enter_context(tc.tile_pool(name="qkv", bufs=2))
        dnp = ctx.enter_context(tc.tile_pool(name="dn", bufs=4))
        rp = ctx.enter_context(tc.tile_pool(name="r", bufs=2))
        ptp = ctx.enter_context(tc.tile_pool(name="pt", bufs=2))
        ytp = ctx.enter_context(tc.tile_pool(name="yt", bufs=2))
        op = ctx.enter_context(tc.tile_pool(name="o", bufs=6))
        # PSUM: st 2 + qkv-acc 1 + transpose 1 + (yt_ps|o_ps) 2 + (den|rec) 2 = 8
        psA = ctx.enter_context(tc.tile_pool(name="psA", bufs=2, space="PSUM"))
        psQ = ctx.enter_context(tc.tile_pool(name="psQ", bufs=1, space="PSUM"))
        psV = ctx.enter_context(tc.tile_pool(name="psV", bufs=2, space="PSUM"))
        psD = ctx.enter_context(tc.tile_pool(name="psD", bufs=2, space="PSUM"))
        psT = ctx.enter_context(tc.tile_pool(name="psT", bufs=1, space="PSUM"))

        ones_col = const.tile([128, 1], f16)
        nc.vector.memset(ones_col, 1.0)

        w_sb = wpool.tile([128, NCC, 6, HD], f16)
        wp_sb = wpool.tile([128, 2, C], f16)

        def emit_qkv_dma(b, tch):
            t0 = b * T + tch * 512
            xt_t = xtp.tile([128, NCC, 512], f16, tag="xt")
            for cc in range(NCC):
                nc.sync.dma_start(
                    xt_t[:, cc, :], xT[cc * 128:(cc + 1) * 128, t0:t0 + 512])
            return xt_t

        def make_qkv_steps(qkv_tiles, xt_t, tch, qtag):
            """QKV-projection work for one 512-token chunk, as a list of
            small closures so it can be drip-fed between attention matmuls."""
            qt, kt, v = qkv_tiles
            steps = []
            acc = {}

            def mm_step(jj, cc):
                if cc == 0:
                    pool = psA if qtag == "psA" else psQ
                    acc[jj] = pool.tile([128, 512], f32, tag=qtag,
                                        name="qk_ps")
                nc.tensor.matmul(acc[jj], w_sb[:, cc, jj, :], xt_t[:, cc, :],
                                 start=(cc == 0), stop=(cc == NCC - 1))
                if cc == NCC - 1:
                    dst = (qt, qt, kt, kt)[jj]
                    nc.vector.tensor_copy(
                        dst[:, jj % 2, tch * 512:(tch + 1) * 512], acc[jj])

            def v_step(tb, cc):
                # V directly in [t, d] orientation: x-tile stationary,
                # both heads' Wv columns streaming — no PE transposes
                if cc == 0:
                    acc[6 + tb] = psT.tile([128, HPC * HD], f32, tag="psT",
                                           name="v_ps")
                nc.tensor.matmul(
                    acc[6 + tb], xt_t[:, cc, tb * 128:(tb + 1) * 128],
                    w_sb[:, cc, 4:6, :],
                    start=(cc == 0), stop=(cc == NCC - 1))
                if cc == NCC - 1:
                    nc.vector.tensor_copy(v[:, tch * 4 + tb, :], acc[6 + tb])

            for jj in range(4):  # q_h0, q_h1, k_h0, k_h1
                for cc in range(NCC):
                    steps.append(lambda jj=jj, cc=cc: mm_step(jj, cc))
            for tb in range(4):
                for cc in range(NCC):
                    steps.append(lambda tb=tb, cc=cc: v_step(tb, cc))
            return steps

        class Feeder:
            def __init__(self):
                self.steps = []
                self.i = 0

            def add(self, steps):
                self.steps.extend(steps)

            def remaining(self):
                return len(self.steps) - self.i

            def take(self, n):
                for _ in range(min(n, self.remaining())):
                    self.steps[self.i]()
                    self.i += 1

        def emit_attn_unit(b, qg, h, qkv_tiles, feeder):
            qt, kt, v = qkv_tiles
            pt_sb = ptp.tile([128, T // 128, 512], f16, tag="pt")
            den_ps = psD.tile([1, 512], f32, tag="psD")
            yt_ps = psV.tile([128, 512], f32, tag="psV")
            nkb = 4 * qg + 4

            def score(kb):
                kk = kb - 4 * qg
                qs = max(0, kk) * 128
                st = psA.tile([128, 512], f32, tag="psA")
                nc.tensor.matmul(
                    st[:, qs:512], kt[:, h, kb * 128:(kb + 1) * 128],
                    qt[:, h, qg * 512 + qs:(qg + 1) * 512],
                    start=True, stop=True)
                return st, qs

            pipe = score(0)
            for kb in range(nkb):
                st, qs = pipe
                nc.scalar.activation(
                    pt_sb[:, kb, qs:512], st[:, qs:512], Exp, scale=SCALE)
                if kb - 4 * qg >= 0:
                    # causal mask: zero the lower triangle of the 128x128
                    # diagonal block (partition k_rel > free q_rel)
                    nc.gpsimd.affine_select(
                        out=pt_sb[:, kb, qs:qs + 128],
                        in_=pt_sb[:, kb, qs:qs + 128],
                        compare_op=mybir.AluOpType.is_ge, fill=0.0,
                        base=0, pattern=[[1, 128]], channel_multiplier=-1)
                if kb + 1 < nkb:
                    pipe = score(kb + 1)    # PE fills ACT-exp latency
                feeder.take(1)              # drip one QKV-projection step
                nc.tensor.matmul(
                    den_ps[0:1, qs:512], ones_col, pt_sb[:, kb, qs:512],
                    start=(kb == 0), stop=(kb == nkb - 1))
                nc.tensor.matmul(
                    yt_ps[:, qs:512], v[:, kb, h * HD:(h + 1) * HD],
                    pt_sb[:, kb, qs:512],
                    start=(kb == 0), stop=(kb == nkb - 1))
            # reciprocal on DVE ([1,512] costs ~3.3us there, but it runs
            # while the QKV flush keeps the PE fed)
            rec_row = dnp.tile([1, 512], f32, tag="rec")
            nc.vector.reciprocal(rec_row, den_ps[0:1, :])
            return rec_row, yt_ps

        def emit_attn_norm(yt, h, state):
            # PE-free normalization: broadcast 1/den across partitions on
            # the (otherwise idle) GPSIMD engine, then scale y^T on DVE
            rec_row, yt_ps = state
            r_sb = rp.tile([128, 512], f32, tag="rsb")
            nc.gpsimd.partition_broadcast(r_sb, rec_row)
            nc.vector.tensor_mul(yt[:, h, :], yt_ps, r_sb)

        def emit_proj(b, qg, yt):
            for tt in range(4):
                for co in range(4):
                    o_ps = psV.tile([128, 512], f32, tag="psV")
                    for jh in range(HPC):
                        nc.tensor.matmul(
                            o_ps, yt[:, jh, tt * 128:(tt + 1) * 128],
                            wp_sb[:, jh, co * 512:(co + 1) * 512],
                            start=(jh == 0), stop=(jh == HPC - 1))
                    o_sb = op.tile([128, 512], f16, tag="osb")
                    if (tt * 4 + co) % 2 == 0:
                        nc.vector.tensor_copy(o_sb, o_ps)
                    else:
                        nc.scalar.copy(o_sb, o_ps)
                    r0 = b * T + qg * 512 + tt * 128
                    nc.sync.dma_start(
                        out[r0:r0 + 128, co * 512:(co + 1) * 512], o_sb)

        def alloc_qkv_tiles():
            qt = qkvp.tile([128, HPC, T], f16, tag="qt")
            kt = qkvp.tile([128, HPC, T], f16, tag="kt")
            v = qkvp.tile([128, T // 128, HPC * HD], f16, tag="v")
            return (qt, kt, v)

        # Prologue: batch 0's QKV, with weight DMAs interleaved with the
        # first x chunk's DMAs so the first matmul starts ~1us in.
        tiles = alloc_qkv_tiles()
        xt0 = xtp.tile([128, NCC, 512], f16, tag="xt")
        for cc in range(NCC):
            nc.sync.dma_start(w_sb[:, cc], wqkv_v[:, cc])
            nc.sync.dma_start(xt0[:, cc, :], xT[cc * 128:(cc + 1) * 128, 0:512])
        nc.sync.dma_start(wp_sb, wproj_v)
        for s in make_qkv_steps(tiles, xt0, 0, "psA"):
            s()
        for tch in range(1, NTCH):
            xt_t = emit_qkv_dma(0, tch)
            for s in make_qkv_steps(tiles, xt_t, tch, "psA"):
                s()

        # chunk k (k=0..11) = batch k//4+1, t-chunk k%4; fed to group k (its
        # xt DMA prefetched one group earlier). After the attention units,
        # ~COVER steps of QKV matmuls run while the DVE reciprocals finish,
        # so the PE never waits on the softmax-denominator chain. Chunk 11
        # (needed only by group 15's units) is held back and dribbled into
        # the otherwise QKV-less batch-3 groups.
        COVER = 20
        feeder = Feeder()
        pending_xt = emit_qkv_dma(1, 0)
        for b in range(B):
            nxt = alloc_qkv_tiles() if b + 1 < B else None
            for qg in range(4):
                k = 4 * b + qg
                if nxt is not None:
                    feeder.add(make_qkv_steps(nxt, pending_xt, qg, "psQ"))
                    if k + 1 < 4 * (B - 1):
                        pending_xt = emit_qkv_dma((k + 1) // 4 + 1, (k + 1) % 4)
                yt = ytp.tile([128, HPC, 512], f16, tag="yt")
                s0 = emit_attn_unit(b, qg, 0, tiles, feeder)
                s1 = emit_attn_unit(b, qg, 1, tiles, feeder)
                feeder.take(COVER)
                if k < 4 * (B - 1) - 1:
                    feeder.take(feeder.remaining())
                emit_attn_norm(yt, 0, s0)
                emit_attn_norm(yt, 1, s1)
                emit_proj(b, qg, yt)
            tiles = nxt

    nc.compile()
    return nc


def _get_nc():
    if "nc" not in _CACHE:
        _CACHE["nc"] = _build_nc()
    return _CACHE["nc"]


def _make_in_maps(x2d, Wqkv, Wproj):
    xT = np.ascontiguousarray(x2d.T).astype(np.float16)  # [C, B*T]
    in_maps = []
    for c in range(N_CORES):
        h0 = c * HPC
        cols = []
        for part in range(3):  # q, k, v blocks of Wqkv columns
            for h in range(HPC):
                j0 = part * C + (h0 + h) * HD
                cols.append(Wqkv[:, j0:j0 + HD])
        wq = np.ascontiguousarray(np.concatenate(cols, axis=1)).astype(np.float16)
        wp = np.ascontiguousarray(
            Wproj[h0 * HD:(h0 + HPC) * HD, :]).astype(np.float16)
        in_maps.append({"xt": xT, "wqkv": wq, "wproj": wp})
    return in_maps


def run_shards(in_maps, trace=False):
    from concourse.bass_utils import run_bass_kernel_spmd
    nc = _get_nc()
    last_err = None
    for _attempt in range(3):
        try:
            return run_bass_kernel_spmd(
                nc, in_maps, core_ids=list(range(N_CORES)), trace=trace)
        except Exception as e:  # transient NRT device errors — retry
            last_err = e
            if "UNAVAILABLE" not in str(e) and "UNRECOVERABLE" not in str(e):
                raise
    raise last_err


def kernel(x, Wqkv, Wproj):
    x = np.asarray(x, dtype=np.float32)
    Wqkv = np.asarray(Wqkv, dtype=np.float32)
    Wproj = np.asarray(Wproj, dtype=np.float32)
    x2d = np.ascontiguousarray(x.reshape(B * T, C))

    in_maps = _make_in_maps(x2d, Wqkv, Wproj)
    res = run_shards(in_maps)

    acc = res.results[0]["out"].astype(np.float32)
    for c in range(1, N_CORES):
        acc += res.results[c]["out"].astype(np.float32)
    return acc.reshape(B, T, C)


# revision 13
# speedup vs baseline: 1.0283x; 1.0115x over previous
"""Causal self-attention (B=4, T=2048, C=2048, H=16) on 8 trn2 NeuronCores.

Sharding: tensor-parallel over heads — 2 heads per core. Every core gets the
full (pre-transposed) activation xT, its 2 heads' slice of Wqkv columns and
Wproj rows, computes a full [B*T, C] partial output (fp16), and the host sums
the 8 partials (the "all-reduce after output projection" done host-side).

Per-core dataflow (all matmuls on PE, fp16 operands):
  xT tiles --DMA--> QKV proj -> Q^T,K^T [d,t] + V [t,d] (via PE transpose)
  S^T = K^T-block.T @ Q^T chunks (PSUM f32) -> exp (ACT) -> P^T fp16 in SBUF
  causal mask: GPSIMD affine_select zeroes the lower triangle of diag blocks
  den = ones^T @ P^T blocks (PE, PSUM acc); y^T = sum_k V_k @ P^T_k (PE)
  rec = exp(-ln(den)) on ACT (both funcs in one table set);
  broadcast rec via PE, scale y^T on DVE  (deferred past independent QKV work
  so the PE FIFO never stalls on the recip chain)
  out_partial = y^T.T @ Wproj-rows -> fp16 -> DMA out

Scheduling: next batch's QKV-projection matmuls are drip-fed between the
attention blocks' dependent matmuls (score depth-2 pipeline) so the PE always
has runnable work during ACT exp latency; long PE-idle gaps would re-throttle
the PE clock (HAM) to 1.2 GHz.
"""
import numpy as np

B, T, C = 4, 2048, 2048
H, HD = 16, 128
N_CORES = 8
HPC = H // N_CORES          # heads per core = 2
SCALE = float(1.0 / np.sqrt(HD))

_CACHE = {}


def _build_nc():
    import concourse.bass as bass
    from concourse import bacc
    import concourse.tile as tile
    import concourse.mybir as mybir
    from contextlib import ExitStack

    f32 = mybir.dt.float32
    f16 = mybir.dt.float16
    Exp = mybir.ActivationFunctionType.Exp

    nc = bacc.Bacc("TRN2", target_bir_lowering=False, debug=False,
                   enable_asserts=True, num_devices=N_CORES)

    # Inputs (per-core shards prepared on host)
    xT = nc.dram_tensor("xt", [C, B * T], f16, kind="ExternalInput").ap()
    wqkv = nc.dram_tensor("wqkv", [C, 6 * HD], f16, kind="ExternalInput").ap()
    wproj = nc.dram_tensor("wproj", [HPC * HD, C], f16, kind="ExternalInput").ap()
    out = nc.dram_tensor("out", [B * T, C], f16, kind="ExternalOutput").ap()

    # DRAM views: c-chunked weights
    wqkv_v = wqkv.rearrange("(cc p) (jj d) -> p cc jj d", p=128, d=HD)  # [128,16,6,128]
    wproj_v = wproj.rearrange("(jh p) c -> p jh c", p=128)              # [128,2,2048]

    NCC = C // 128        # 16 contraction chunks
    NTCH = T // 512       # 4 t-chunks per batch

    with tile.TileContext(nc) as tc, ExitStack() as ctx:
        const = ctx.enter_context(tc.tile_pool(name="const", bufs=1))
        wpool = ctx.enter_context(tc.tile_pool(name="w", bufs=1))
        xtp = ctx.enter_context(tc.tile_pool(name="xt", bufs=2))
        qkvp = ctx.enter_context(tc.tile_pool(name="qkv", bufs=2))
        dnp = ctx.enter_context(tc.tile_pool(name="dn", bufs=4))
        rp = ctx.enter_context(tc.tile_pool(name="r", bufs=2))
        ptp = ctx.enter_context(tc.tile_pool(name="pt", bufs=2))
        ytp = ctx.enter_context(tc.tile_pool(name="yt", bufs=2))
        op = ctx.enter_context(tc.tile_pool(name="o", bufs=6))
        # PSUM: st 2 + qkv-acc 1 + transpose 1 + (yt_ps|o_ps) 2 + (den|rec) 2 = 8
        psA = ctx.enter_context(tc.tile_pool(name="psA", bufs=2, space="PSUM"))
        psQ = ctx.enter_context(tc.tile_pool(name="psQ", bufs=1, space="PSUM"))
        psV = ctx.enter_context(tc.tile_pool(name="psV", bufs=2, space="PSUM"))
        psD = ctx.enter_context(tc.tile_pool(name="psD", bufs=2, space="PSUM"))
        psT = ctx.enter_context(tc.tile_pool(name="psT", bufs=1, space="PSUM"))

        ones_col = const.tile([128, 1], f16)
        nc.vector.memset(ones_col, 1.0)

        w_sb = wpool.tile([128, NCC, 6, HD], f16)
        wp_sb = wpool.tile([128, 2, C], f16)

        def emit_qkv_dma(b, tch):
            t0 = b * T + tch * 512
            xt_t = xtp.tile([128, NCC, 512], f16, tag="xt")
            for cc in range(NCC):
                nc.sync.dma_start(
                    xt_t[:, cc, :], xT[cc * 128:(cc + 1) * 128, t0:t0 + 512])
            return xt_t

        def make_qkv_steps(qkv_tiles, xt_t, tch, qtag):
            """QKV-projection work for one 512-token chunk, as a list of
            small closures so it can be drip-fed between attention matmuls."""
            qt, kt, v = qkv_tiles
            steps = []
            acc = {}

            def mm_step(jj, cc):
                if cc == 0:
                    pool = psA if qtag == "psA" else psQ
                    acc[jj] = pool.tile([128, 512], f32, tag=qtag,
                                        name="qk_ps")
                nc.tensor.matmul(acc[jj], w_sb[:, cc, jj, :], xt_t[:, cc, :],
                                 start=(cc == 0), stop=(cc == NCC - 1))
                if cc == NCC - 1:
                    dst = (qt, qt, kt, kt)[jj]
                    nc.vector.tensor_copy(
                        dst[:, jj % 2, tch * 512:(tch + 1) * 512], acc[jj])

            def v_step(tb, cc):
                # V directly in [t, d] orientation: x-tile stationary,
                # both heads' Wv columns streaming — no PE transposes
                if cc == 0:
                    acc[6 + tb] = psT.tile([128, HPC * HD], f32, tag="psT",
                                           name="v_ps")
                nc.tensor.matmul(
                    acc[6 + tb], xt_t[:, cc, tb * 128:(tb + 1) * 128],
                    w_sb[:, cc, 4:6, :],
                    start=(cc == 0), stop=(cc == NCC - 1))
                if cc == NCC - 1:
                    nc.vector.tensor_copy(v[:, tch * 4 + tb, :], acc[6 + tb])

            for jj in range(4):  # q_h0, q_h1, k_h0, k_h1
                for cc in range(NCC):
                    steps.append(lambda jj=jj, cc=cc: mm_step(jj, cc))
            for tb in range(4):
                for cc in range(NCC):
                    steps.append(lambda tb=tb, cc=cc: v_step(tb, cc))
            return steps

        class Feeder:
            def __init__(self):
                self.steps = []
                self.i = 0

            def add(self, steps):
                self.steps.extend(steps)

            def remaining(self):
                return len(self.steps) - self.i

            def take(self, n):
                for _ in range(min(n, self.remaining())):
                    self.steps[self.i]()
                    self.i += 1

        def emit_attn_unit(b, qg, h, qkv_tiles, feeder):
            qt, kt, v = qkv_tiles
            pt_sb = ptp.tile([128, T // 128, 512], f16, tag="pt")
            den_ps = psD.tile([1, 512], f32, tag="psD")
            yt_ps = psV.tile([128, 512], f32, tag="psV")
            nkb = 4 * qg + 4

            def score(kb):
                kk = kb - 4 * qg
                qs = max(0, kk) * 128
                st = psA.tile([128, 512], f32, tag="psA")
                nc.tensor.matmul(
                    st[:, qs:512], kt[:, h, kb * 128:(kb + 1) * 128],
                    qt[:, h, qg * 512 + qs:(qg + 1) * 512],
                    start=True, stop=True)
                return st, qs

            pipe = score(0)
            for kb in range(nkb):
                st, qs = pipe
                nc.scalar.activation(
                    pt_sb[:, kb, qs:512], st[:, qs:512], Exp, scale=SCALE)
                if kb - 4 * qg >= 0:
                    # causal mask: zero the lower triangle of the 128x128
                    # diagonal block (partition k_rel > free q_rel)
                    nc.gpsimd.affine_select(
                        out=pt_sb[:, kb, qs:qs + 128],
                        in_=pt_sb[:, kb, qs:qs + 128],
                        compare_op=mybir.AluOpType.is_ge, fill=0.0,
                        base=0, pattern=[[1, 128]], channel_multiplier=-1)
                if kb + 1 < nkb:
                    pipe = score(kb + 1)    # PE fills ACT-exp latency
                feeder.take(1)              # drip one QKV-projection step
                nc.tensor.matmul(
                    den_ps[0:1, qs:512], ones_col, pt_sb[:, kb, qs:512],
                    start=(kb == 0), stop=(kb == nkb - 1))
                nc.tensor.matmul(
                    yt_ps[:, qs:512], v[:, kb, h * HD:(h + 1) * HD],
                    pt_sb[:, kb, qs:512],
                    start=(kb == 0), stop=(kb == nkb - 1))
            # reciprocal on DVE ([1,512] costs ~3.3us there, but it runs
            # while the QKV flush keeps the PE fed)
            rec_row = dnp.tile([1, 512], f32, tag="rec")
            nc.vector.reciprocal(rec_row, den_ps[0:1, :])
            return rec_row, yt_ps

        def emit_attn_norm(yt, h, state):
            # PE-free normalization: broadcast 1/den across partitions on
            # the (otherwise idle) GPSIMD engine, then scale y^T on DVE
            rec_row, yt_ps = state
            r_sb = rp.tile([128, 512], f32, tag="rsb")
            nc.gpsimd.partition_broadcast(r_sb, rec_row)
            nc.vector.tensor_mul(yt[:, h, :], yt_ps, r_sb)

        def emit_proj(b, qg, yt):
            for tt in range(4):
                for co in range(4):
                    o_ps = psV.tile([128, 512], f32, tag="psV")
                    for jh in range(HPC):
                        nc.tensor.matmul(
                            o_ps, yt[:, jh, tt * 128:(tt + 1) * 128],
                            wp_sb[:, jh, co * 512:(co + 1) * 512],
                            start=(jh == 0), stop=(jh == HPC - 1))
                    # PSUM->SBUF cast stays on DVE: putting any of these on
                    # ACT head-of-line-blocks the next unit's exps behind
                    # sem-waits for the proj matmuls
                    o_sb = op.tile([128, 512], f16, tag="osb")
                    nc.vector.tensor_copy(o_sb, o_ps)
                    r0 = b * T + qg * 512 + tt * 128
                    nc.sync.dma_start(
                        out[r0:r0 + 128, co * 512:(co + 1) * 512], o_sb)

        def alloc_qkv_tiles():
            qt = qkvp.tile([128, HPC, T], f16, tag="qt")
            kt = qkvp.tile([128, HPC, T], f16, tag="kt")
            v = qkvp.tile([128, T // 128, HPC * HD], f16, tag="v")
            return (qt, kt, v)

        # Prologue: batch 0's QKV, with weight DMAs interleaved with the
        # first x chunk's DMAs so the first matmul starts ~1us in.
        tiles = alloc_qkv_tiles()
        xt0 = xtp.tile([128, NCC, 512], f16, tag="xt")
        for cc in range(NCC):
            nc.sync.dma_start(w_sb[:, cc], wqkv_v[:, cc])
            nc.sync.dma_start(xt0[:, cc, :], xT[cc * 128:(cc + 1) * 128, 0:512])
        nc.sync.dma_start(wp_sb, wproj_v)
        for s in make_qkv_steps(tiles, xt0, 0, "psA"):
            s()
        for tch in range(1, NTCH):
            xt_t = emit_qkv_dma(0, tch)
            for s in make_qkv_steps(tiles, xt_t, tch, "psA"):
                s()

        # chunk k (k=0..11) = batch k//4+1, t-chunk k%4; fed to group k (its
        # xt DMA prefetched one group earlier). After the attention units,
        # ~COVER steps of QKV matmuls run while the DVE reciprocals finish,
        # so the PE never waits on the softmax-denominator chain. Chunk 11
        # (needed only by group 15's units) is held back and dribbled into
        # the otherwise QKV-less batch-3 groups.
        COVER = 26
        feeder = Feeder()
        pending_xt = emit_qkv_dma(1, 0)
        for b in range(B):
            nxt = alloc_qkv_tiles() if b + 1 < B else None
            for qg in range(4):
                k = 4 * b + qg
                if nxt is not None:
                    feeder.add(make_qkv_steps(nxt, pending_xt, qg, "psQ"))
                    if k + 1 < 4 * (B - 1):
                        pending_xt = emit_qkv_dma((k + 1) // 4 + 1, (k + 1) % 4)
                yt = ytp.tile([128, HPC, 512], f16, tag="yt")
                s0 = emit_attn_unit(b, qg, 0, tiles, feeder)
                s1 = emit_attn_unit(b, qg, 1, tiles, feeder)
                feeder.take(COVER)
                if k < 4 * (B - 1) - 1:
                    feeder.take(feeder.remaining())
                emit_attn_norm(yt, 0, s0)
                emit_attn_norm(yt, 1, s1)
                emit_proj(b, qg, yt)
            tiles = nxt

    nc.compile()
    return nc


def _get_nc():
    if "nc" not in _CACHE:
        _CACHE["nc"] = _build_nc()
    return _CACHE["nc"]


def _make_in_maps(x2d, Wqkv, Wproj):
    xT = np.ascontiguousarray(x2d.T).astype(np.float16)  # [C, B*T]
    in_maps = []
    for c in range(N_CORES):
        h0 = c * HPC
        cols = []
        for part in range(3):  # q, k, v blocks of Wqkv columns
            for h in range(HPC):
                j0 = part * C + (h0 + h) * HD
                cols.append(Wqkv[:, j0:j0 + HD])
        wq = np.ascontiguousarray(np.concatenate(cols, axis=1)).astype(np.float16)
        wp = np.ascontiguousarray(
            Wproj[h0 * HD:(h0 + HPC) * HD, :]).astype(np.float16)
        in_maps.append({"xt": xT, "wqkv": wq, "wproj": wp})
    return in_maps


def run_shards(in_maps, trace=False):
    from concourse.bass_utils import run_bass_kernel_spmd
    nc = _get_nc()
    last_err = None
    for _attempt in range(3):
        try:
            return run_bass_kernel_spmd(
                nc, in_maps, core_ids=list(range(N_CORES)), trace=trace)
        except Exception as e:  # transient NRT device errors — retry
            last_err = e
            if "UNAVAILABLE" not in str(e) and "UNRECOVERABLE" not in str(e):
                raise
    raise last_err


def kernel(x, Wqkv, Wproj):
    x = np.asarray(x, dtype=np.float32)
    Wqkv = np.asarray(Wqkv, dtype=np.float32)
    Wproj = np.asarray(Wproj, dtype=np.float32)
    x2d = np.ascontiguousarray(x.reshape(B * T, C))

    in_maps = _make_in_maps(x2d, Wqkv, Wproj)
    res = run_shards(in_maps)

    acc = res.results[0]["out"].astype(np.float32)
    for c in range(1, N_CORES):
        acc += res.results[c]["out"].astype(np.float32)
    return acc.reshape(B, T, C)
